# revision 23
# baseline (speedup 1.0000x reference)
"""Trainium2 Bass kernel for nn_AttentionWithMemory (local-window MHA block).

Sharding: data-parallel over batch — one batch element per NeuronCore (8 cores).
Per core: x_b [1024,1024] -> qkv in-proj -> 16-head local attention (window 32,
band +-16) -> out-proj -> out_b [1024,1024].

v2 design (transpose-free attention; all matmuls bf16, fp32 PSUM accumulate):
  - host pre-transposes/casts: xT [D,S] bf16, w_inT [D,3D] bf16, w_outT [D,D]
    bf16 (matmul contraction dim on SBUF partitions for both operands).
  - qT,kT feature-major [128=2 heads x 64, tokens]; v token-major with a ones
    column appended per head ([128, 16, 65]).
  - scores are computed TRANSPOSED directly (S^T [keys, q]) with K=64 matmuls
    (even head on array rows 0-63, odd head on rows 64-127 -> row-group
    concurrency), 4 heads packed per PSUM bank; main 128-key slice + 32-key
    wing per 128-query tile.
  - exp on scalar engine (scale=1/8, no max subtraction: scores ~ N(0,1));
    band masking applied POST-exp on the bf16 SBUF tiles via gpsimd
    affine_select (two slope-1 predicates main, one wing) — no mask tensor,
    no DVE mask cost, and the PE never runs transposes.
  - ctx^T = [V|1]^T @ P^T per head (M=65): row 64 of the ctx PSUM is the
    softmax denominator l for free. reciprocal (DVE) -> partition_broadcast
    (gpsimd) -> per-column normalize on eviction (DVE tensor_mul).
  - even head evicts straight into ctxT rows 0-63; odd head goes through a
    staging tile + SBUF->SBUF DMA partition shift to rows 64-127.
  - out-proj token-major (lhsT=ctxT, rhs=w_outT), bias added on evict.
  - dense projection / out-proj matmul groups interleaved between attention
    packs to keep the PE HAM-warm end to end.
"""

import os
import sys

sys.path.insert(0, "/opt/trn_rl_repo")

import numpy as np

B, S, D = 8, 1024, 1024
H, HD = 16, 64
P = 128
NT = S // P  # 8 query/token tiles
N_CORES = 8

_CACHE = {}


def _build_nc():
    # bisect switches: V2_BCAST=gps|pe, V2_SEL=pack|chunk,
    # V2_LV: 1=scores+exp+sel only, 2=+ctx (no norm), 3=full
    bcast_mode = os.environ.get("V2_BCAST", "gps")
    sel_mode = os.environ.get("V2_SEL", "chunk")
    attn_lv = int(os.environ.get("V2_LV", "3"))
    m65 = os.environ.get("V2_M65", "1") == "1"
    do_shift = os.environ.get("V2_SHIFT", "1") == "1"
    do_wing = os.environ.get("V2_WING", "1") == "1"
    import concourse.bacc as bacc
    import concourse.mybir as mybir
    import concourse.tile as tile

    dt = mybir.dt
    f32, bf16 = dt.float32, dt.bfloat16
    Act = mybir.ActivationFunctionType
    Alu = mybir.AluOpType

    nc = bacc.Bacc("TRN2", target_bir_lowering=False, debug=False,
                   num_devices=N_CORES)

    xt_d = nc.dram_tensor("xt", [D, S], bf16, kind="ExternalInput").ap()
    wi_d = nc.dram_tensor("w_int", [D, 3 * D], bf16, kind="ExternalInput").ap()
    wo_d = nc.dram_tensor("w_outt", [D, D], bf16, kind="ExternalInput").ap()
    bin_d = nc.dram_tensor("b_in_t", [P, 16], f32, kind="ExternalInput").ap()
    bv_d = nc.dram_tensor("bv_bc", [P, D], bf16, kind="ExternalInput").ap()
    bo_d = nc.dram_tensor("bo_bc", [P, D], bf16, kind="ExternalInput").ap()
    out_d = nc.dram_tensor("out", [S, D], f32, kind="ExternalOutput").ap()
    dbg = os.environ.get("V2_DBG", "0") == "1"
    if dbg:
        dbg_p4 = nc.dram_tensor("dbg_p4", [P, 4, P], bf16, kind="ExternalOutput").ap()
        dbg_pw = nc.dram_tensor("dbg_pw", [32, 4, P], bf16, kind="ExternalOutput").ap()
        dbg_r = nc.dram_tensor("dbg_r", [65, 2, P], f32, kind="ExternalOutput").ap()
        dbg_rbc = nc.dram_tensor("dbg_rbc", [64, 2, P], f32, kind="ExternalOutput").ap()
        dbg_ctx = nc.dram_tensor("dbg_ctx", [P, 512], bf16, kind="ExternalOutput").ap()


    with tile.TileContext(nc) as tc:
        with (
            tc.tile_pool(name="const", bufs=1) as cpool,
            tc.tile_pool(name="acts", bufs=1) as apool,
            tc.tile_pool(name="pmain", bufs=4) as pmpool,
            tc.tile_pool(name="pwing", bufs=4) as pwpool,
            tc.tile_pool(name="lr", bufs=2) as lrpool,
            tc.tile_pool(name="r0", bufs=2) as r0pool,
            tc.tile_pool(name="rbc", bufs=2) as rbcpool,
            tc.tile_pool(name="stage", bufs=3) as stgpool,
            tc.tile_pool(name="outsb", bufs=2) as outpool,
            tc.tile_pool(name="ps_mm", bufs=2, space="PSUM") as ps_mm,
            tc.tile_pool(name="ps_st", bufs=2, space="PSUM") as ps_st,
            tc.tile_pool(name="ps_wg", bufs=2, space="PSUM") as ps_wg,
            tc.tile_pool(name="ps_pp", bufs=2, space="PSUM") as ps_pp,
        ):
            # ---- persistent SBUF tensors ----
            xt = [cpool.tile([P, S], bf16, tag=f"xt{i}", name=f"xt{i}") for i in range(NT)]
            wi = [cpool.tile([P, 3 * D], bf16, tag=f"wi{i}", name=f"wi{i}") for i in range(NT)]
            wo = [cpool.tile([P, D], bf16, tag=f"wo{i}", name=f"wo{i}") for i in range(NT)]
            bint = cpool.tile([P, 16], f32, tag="bint", name="bint")
            bv = cpool.tile([P, 16, 64], bf16, tag="bv", name="bv")
            bo = cpool.tile([P, D], bf16, tag="bo", name="bo")

            kT = [apool.tile([P, S], bf16, tag=f"kT{h}", name=f"kT{h}") for h in range(H)]
            qT = [apool.tile([P, S], bf16, tag=f"qT{i}", name=f"qT{i}") for i in range(NT)]
            v = [apool.tile([P, 16, 72], bf16, tag=f"v{i}", name=f"v{i}") for i in range(NT)]
            voff = [None] + [apool.tile([P, 16, 72], bf16, tag=f"voff{j}", name=f"voff{j}")
                             for j in range(1, NT + 1)]
            ctxT = [apool.tile([P, S], bf16, tag=f"ctxT{i}", name=f"ctxT{i}") for i in range(NT)]

            zfill = nc.gpsimd.to_reg(0.0)
            ones_r = cpool.tile([65, 2, P], f32, tag="ones_r", name="ones_r")
            nc.vector.memset(ones_r[64:65, :, :], 1.0)
            if attn_lv < 2:
                for i in range(NT):
                    nc.vector.memset(ctxT[i], 0.0)

            def band_select(ap, base, cm, step, reps, width):
                """keep where base + cm*partition + step*inner_idx >= 0"""
                if sel_mode == "pack":
                    nc.gpsimd.affine_select(
                        out=ap, in_=ap, compare_op=Alu.is_ge, fill=zfill,
                        base=base, channel_multiplier=cm,
                        pattern=[[0, reps], [step, width]],
                    )
                else:
                    for c in range(reps):
                        nc.gpsimd.affine_select(
                            out=ap[:, c, :], in_=ap[:, c, :],
                            compare_op=Alu.is_ge, fill=zfill,
                            base=base, channel_multiplier=cm,
                            pattern=[[step, width]],
                        )

            # ---- loads ----
            for i in range(NT):
                nc.sync.dma_start(out=xt[i], in_=xt_d[i * P:(i + 1) * P, :])
                nc.sync.dma_start(out=wi[i][:, 2 * D:3 * D],
                                  in_=wi_d[i * P:(i + 1) * P, 2 * D:3 * D])
            for i in range(NT):
                nc.sync.dma_start(out=wi[i][:, 0:2 * D],
                                  in_=wi_d[i * P:(i + 1) * P, 0:2 * D])
            nc.sync.dma_start(out=bint, in_=bin_d)
            nc.sync.dma_start(out=bv, in_=bv_d)
            for i in range(NT):
                nc.vector.memset(v[i][:, :, 64:72], 1.0)
            for h in range(H):
                nc.gpsimd.memset(kT[h][(1 - h % 2) * 64:(2 - h % 2) * 64, :], 0.0)
            for i in range(NT):
                nc.sync.dma_start(out=wo[i], in_=wo_d[i * P:(i + 1) * P, :])
            nc.sync.dma_start(out=bo, in_=bo_d)

            # ---- projection emitters (interleaved with attention below) ----
            def emit_v(st, nh):
                ps = ps_mm.tile([P, 512], f32, tag="mm", name="mmps")
                for dc in range(NT):
                    nc.tensor.matmul(
                        ps,
                        lhsT=xt[dc][:, st * P:(st + 1) * P],
                        rhs=wi[dc][:, 2 * D + nh * 512: 2 * D + (nh + 1) * 512],
                        start=(dc == 0), stop=(dc == NT - 1),
                    )
                nc.vector.tensor_add(v[st][:, nh * 8:(nh + 1) * 8, 0:64],
                                     ps, bv[:, nh * 8:(nh + 1) * 8, :])

            def emit_voff(j):
                if j < NT:
                    nc.sync.dma_start(out=voff[j][0:16, :, :], in_=v[j - 1][112:128, :, :])
                    nc.sync.dma_start(out=voff[j][16:128, :, :], in_=v[j][0:112, :, :])
                else:
                    nc.vector.memset(voff[NT][:, :, :], 0.0)
                    nc.sync.dma_start(out=voff[NT][0:16, :, :], in_=v[NT - 1][112:128, :, :])

            def emit_kq(split_heads, dst, fbase, bias_col, fc, nh):
                ps = ps_mm.tile([P, 512], f32, tag="mm", name="mmps")
                for dc in range(NT):
                    nc.tensor.matmul(
                        ps,
                        lhsT=wi[dc][:, fbase + fc * P: fbase + (fc + 1) * P],
                        rhs=xt[dc][:, nh * 512:(nh + 1) * 512],
                        start=(dc == 0), stop=(dc == NT - 1),
                    )
                bia = bint[:, bias_col + fc: bias_col + fc + 1]
                if split_heads:
                    for hh in range(2):
                        sl = slice(hh * 64, (hh + 1) * 64)
                        nc.vector.tensor_scalar(
                            out=dst[2 * fc + hh][sl, nh * 512:(nh + 1) * 512],
                            in0=ps[sl, :], scalar1=bia[sl, :],
                            scalar2=None, op0=Alu.add,
                        )
                else:
                    nc.vector.tensor_scalar(
                        out=dst[fc][:, nh * 512:(nh + 1) * 512],
                        in0=ps, scalar1=bia, scalar2=None, op0=Alu.add,
                    )

            # ---- attention per query tile, transpose-free ----
            def attention_tile(t, fillers=()):
                fillers = list(fillers)
                kst = 0 if t == 0 else t * P - 16
                vm = v[0] if t == 0 else voff[t]
                vc = v[1] if t == 0 else voff[t + 1]
                # local band: key j (partition), query i (free); t>0 windows
                # start at tP-16 so allowed is 0<=j-i<=32; t=0 is |j-i|<=16.
                b_lo = 16 if t == 0 else 0    # keep j - i + b_lo >= 0
                b_hi = 16 if t == 0 else 32   # keep i - j + b_hi >= 0
                b_wg = -112 if t == 0 else -96  # wing: keep i - jw + b_wg >= 0
                p4s, pws = [], []
                # t=7's wing has only 16 valid keys: run a 16-row wing
                # matmul, pre-zero the wing P tile, exp/select rows 0:16.
                wrows = 16 if t == NT - 1 else 32
                for pk in range(4):
                    stp = ps_st.tile([P, 4, P], f32, tag="st", name="stps")
                    wgp = ps_wg.tile([32, 4, P], f32, tag="wg", name="wgps")
                    # one start=True per PSUM bank: start marks the whole 2KB
                    # bank row (per written partition) pending-zero, so later
                    # chunks must use start=False (they overwrite their own
                    # pending bytes, accumulate nothing).
                    for hh in range(4):
                        h = pk * 4 + hh
                        nc.tensor.matmul(
                            stp[:, hh, :],
                            lhsT=kT[h][:, kst:kst + 128],
                            rhs=qT[h // 2][:, t * P:(t + 1) * P],
                            start=True, stop=True,
                            skip_group_check=True,
                        )
                        nc.tensor.matmul(
                            wgp[0:wrows, hh, :],
                            lhsT=kT[h][:, kst + 128:kst + 128 + wrows],
                            rhs=qT[h // 2][:, t * P:(t + 1) * P],
                            start=True, stop=True,
                            skip_group_check=True,
                        )
                    p4 = pmpool.tile([P, 4, P], bf16, tag="p4", name="p4")
                    nc.scalar.activation(p4, stp, Act.Exp, scale=0.125)
                    band_select(p4, b_lo, 1, -1, 4, P)
                    band_select(p4, b_hi, -1, 1, 4, P)
                    pw = pwpool.tile([32, 4, P], bf16, tag="pw", name="pw")
                    if wrows < 32:
                        nc.vector.memset(pw, 0.0)
                    nc.scalar.activation(pw[0:wrows, :, :], wgp[0:wrows, :, :],
                                         Act.Exp, scale=0.125)
                    band_select(pw[0:wrows, :, :], b_wg, -1, 1, 4, P)
                    if dbg and t == 0 and pk == 0:
                        nc.sync.dma_start(out=dbg_p4, in_=p4)
                        nc.sync.dma_start(out=dbg_pw, in_=pw)
                    p4s.append(p4)
                    pws.append(pw)
                    if fillers:
                        fillers.pop(0)()
                if attn_lv < 2:
                    while fillers:
                        fillers.pop(0)()
                    return
                M = 65 if m65 else 64
                for hp in range(NT):
                    h0, h1 = 2 * hp, 2 * hp + 1
                    pk, c0 = h0 // 4, h0 % 4
                    pp = ps_pp.tile([65, 2, P], f32, tag="pp", name="pps")
                    for cc, hx in ((0, h0), (1, h1)):
                        nc.tensor.matmul(
                            pp[0:M, cc, :], lhsT=vm[:, hx, 0:M],
                            rhs=p4s[pk][:, c0 + cc, :],
                            start=True, stop=(not do_wing),
                            skip_group_check=True,
                        )
                        if do_wing:
                            nc.tensor.matmul(
                                pp[0:M, cc, :], lhsT=vc[0:32, hx, 0:M],
                                rhs=pws[pk][:, c0 + cc, :],
                                start=False, stop=True, skip_group_check=True,
                            )
                    if attn_lv < 3:
                        nc.vector.tensor_copy(ctxT[hp][0:64, t * P:(t + 1) * P],
                                              pp[0:64, 0, :])
                        stg = stgpool.tile([64, P], bf16, tag="stg", name="stg")
                        nc.vector.tensor_copy(stg, pp[0:64, 1, :])
                        if do_shift:
                            nc.sync.dma_start(
                                out=ctxT[hp][64:128, t * P:(t + 1) * P], in_=stg)
                        if hp % 2 == 1 and fillers:
                            fillers.pop(0)()
                        continue
                    r = lrpool.tile([65, 2, P], f32, tag="r", name="r_t")
                    nc.vector.reciprocal(r[64:65, :, :], pp[64:65, :, :])
                    if dbg and t == 0 and hp == 0:
                        nc.vector.tensor_copy(r[0:64, :, :], pp[0:64, :, :])
                        nc.sync.dma_start(out=dbg_r, in_=r)
                    if bcast_mode == "gps":
                        # HW partition_broadcast reads the tile's physical
                        # partition 0 (AP offset ignored): shift r down first.
                        r0 = r0pool.tile([1, 2, P], f32, tag="r0", name="r0")
                        nc.sync.dma_start(out=r0, in_=r[64:65, :, :])
                        rbc = rbcpool.tile([64, 2, P], f32, tag="rbc", name="rbc")
                        nc.gpsimd.partition_broadcast(rbc, r0, channels=64)
                        if dbg and t == 0 and hp == 0:
                            nc.sync.dma_start(out=dbg_rbc, in_=rbc)
                        rb0, rb1 = rbc[:, 0, :], rbc[:, 1, :]
                    else:
                        # PE broadcast: rbc[p, q] = ones[64] (x) r[q], K=1,
                        # then evict to SBUF (DVE can read only one PSUM input)
                        rbp = ps_st.tile([P, 4, P], f32, tag="st", name="rbcps")
                        nc.tensor.matmul(
                            rbp[0:64, 0:2, :], lhsT=ones_r[64:65, 0, 0:64],
                            rhs=r[64:65, :, :],
                            start=True, stop=True, skip_group_check=True,
                        )
                        rbc = rbcpool.tile([64, 2, P], f32, tag="rbc", name="rbc")
                        nc.scalar.copy(rbc, rbp[0:64, 0:2, :])
                        rb0, rb1 = rbc[:, 0, :], rbc[:, 1, :]
                    nc.vector.tensor_mul(ctxT[hp][0:64, t * P:(t + 1) * P],
                                         pp[0:64, 0, :], rb0)
                    stg = stgpool.tile([64, P], bf16, tag="stg", name="stg")
                    nc.vector.tensor_mul(stg, pp[0:64, 1, :], rb1)
                    if do_shift:
                        nc.sync.dma_start(out=ctxT[hp][64:128, t * P:(t + 1) * P],
                                          in_=stg)
                    if hp % 2 == 1 and fillers:
                        fillers.pop(0)()

            def emit_op(st, nh):
                ps = ps_mm.tile([P, 512], f32, tag="mm", name="mmps")
                for fc in range(NT):
                    nc.tensor.matmul(
                        ps,
                        lhsT=ctxT[fc][:, st * P:(st + 1) * P],
                        rhs=wo[fc][:, nh * 512:(nh + 1) * 512],
                        start=(fc == 0), stop=(fc == NT - 1),
                    )
                o_sb = outpool.tile([P, 512], f32, tag="o", name="o_sb")
                nc.vector.tensor_add(o_sb, ps, bo[:, nh * 512:(nh + 1) * 512])
                nc.sync.dma_start(
                    out=out_d[st * P:(st + 1) * P, nh * 512:(nh + 1) * 512],
                    in_=o_sb)

            # ---- interleaved schedule: dense projection/out-proj groups are
            #      injected between attention packs to keep the PE array warm
            #      and fill cross-engine stalls ----
            def F(fn, *a):
                return lambda: fn(*a)

            for st in range(4):
                for nh in range(2):
                    emit_v(st, nh)
            for j in range(1, 4):
                emit_voff(j)
            for fc in range(NT):
                emit_kq(True, kT, D, 8, fc, 0)
            for fc in range(NT):
                emit_kq(False, qT, 0, 0, fc, 0)

            attention_tile(0, [F(emit_v, 4, 0), F(emit_v, 4, 1),
                               F(emit_kq, True, kT, D, 8, 0, 1),
                               F(emit_kq, True, kT, D, 8, 1, 1)])
            emit_voff(4)
            attention_tile(1, [F(emit_kq, True, kT, D, 8, 2, 1),
                               F(emit_kq, True, kT, D, 8, 3, 1),
                               F(emit_kq, True, kT, D, 8, 4, 1),
                               F(emit_kq, True, kT, D, 8, 5, 1),
                               F(emit_v, 5, 0), F(emit_v, 5, 1)])
            emit_voff(5)
            attention_tile(2, [F(emit_kq, True, kT, D, 8, 6, 1),
                               F(emit_kq, True, kT, D, 8, 7, 1),
                               F(emit_kq, False, qT, 0, 0, 0, 1),
                               F(emit_kq, False, qT, 0, 0, 1, 1),
                               F(emit_v, 6, 0), F(emit_v, 6, 1)])
            emit_voff(6)
            attention_tile(3, [F(emit_kq, False, qT, 0, 0, 2, 1),
                               F(emit_kq, False, qT, 0, 0, 3, 1),
                               F(emit_kq, False, qT, 0, 0, 4, 1),
                               F(emit_kq, False, qT, 0, 0, 5, 1),
                               F(emit_kq, False, qT, 0, 0, 6, 1),
                               F(emit_kq, False, qT, 0, 0, 7, 1),
                               F(emit_v, 7, 0), F(emit_v, 7, 1)])
            emit_voff(7)
            emit_voff(8)
            if dbg:
                nc.sync.dma_start(out=dbg_ctx, in_=ctxT[0][:, 0:512])
            attention_tile(4, [F(emit_op, 0, 0), F(emit_op, 0, 1),
                               F(emit_op, 1, 0), F(emit_op, 1, 1)])
            attention_tile(5, [F(emit_op, 2, 0), F(emit_op, 2, 1),
                               F(emit_op, 3, 0), F(emit_op, 3, 1)])
            attention_tile(6, [F(emit_op, 4, 0), F(emit_op, 4, 1),
                               F(emit_op, 5, 0), F(emit_op, 5, 1)])
            attention_tile(7, [F(emit_op, 6, 0), F(emit_op, 6, 1)])
            emit_op(7, 0)
            emit_op(7, 1)

    nc.compile()
    return nc


def _get_nc():
    if "nc" not in _CACHE:
        _CACHE["nc"] = _build_nc()
    return _CACHE["nc"]


def _prep_inputs(x, w_in, b_in, w_out, b_out, mask):
    import ml_dtypes
    bf16 = ml_dtypes.bfloat16

    x = np.asarray(x, np.float32)
    w_in = np.asarray(w_in, np.float32)
    b_in = np.asarray(b_in, np.float32)
    w_out = np.asarray(w_out, np.float32)
    b_out = np.asarray(b_out, np.float32)

    w_int = np.ascontiguousarray(w_in.T).astype(bf16)          # [D, 3D]
    w_outt = np.ascontiguousarray(w_out.T).astype(bf16)        # [D, D]
    # q,k bias per-partition layout: col c (= global feature chunk), row p
    b_qk = b_in[:2 * D].reshape(16, P).T.astype(np.float32).copy()  # [128,16]
    bv_bc = np.broadcast_to(b_in[2 * D:], (P, D)).astype(bf16).copy()
    bo_bc = np.broadcast_to(b_out, (P, D)).astype(bf16).copy()

    in_maps = []
    for b in range(B):
        xt = np.ascontiguousarray(x[b].T).astype(bf16)         # [D, S]
        in_maps.append({
            "xt": xt, "w_int": w_int, "w_outt": w_outt,
            "b_in_t": b_qk, "bv_bc": bv_bc, "bo_bc": bo_bc,
        })
    return in_maps


def run(x, w_in, b_in, w_out, b_out, mask, trace=False):
    from concourse.bass_utils import run_bass_kernel_spmd
    nc = _get_nc()
    in_maps = _prep_inputs(x, w_in, b_in, w_out, b_out, mask)
    res = run_bass_kernel_spmd(nc, in_maps, list(range(N_CORES)), trace=trace)
    out = np.stack([np.asarray(res.results[b]["out"], np.float32)
                    for b in range(B)])
    return out, res


def kernel(x, w_in, b_in, w_out, b_out, mask):
    out, _ = run(x, w_in, b_in, w_out, b_out, mask)
    return out


# revision 25
# speedup vs baseline: 1.1485x; 1.1485x over previous
"""Trainium2 Bass kernel for nn_AttentionWithMemory (local-window MHA block).

Sharding: data-parallel over batch — one batch element per NeuronCore (8 cores).
Per core: x_b [1024,1024] -> qkv in-proj -> 16-head local attention (window 32,
band +-16) -> out-proj -> out_b [1024,1024].

v2 design (transpose-free attention; all matmuls bf16, fp32 PSUM accumulate):
  - host pre-transposes/casts: xT [D,S] bf16, w_inT [D,3D] bf16, w_outT [D,D]
    bf16 (matmul contraction dim on SBUF partitions for both operands).
  - qT,kT feature-major [128=2 heads x 64, tokens]; v token-major with a ones
    column appended per head ([128, 16, 65]).
  - scores are computed TRANSPOSED directly (S^T [keys, q]) with K=64 matmuls
    (even head on array rows 0-63, odd head on rows 64-127 -> row-group
    concurrency), 4 heads packed per PSUM bank; main 128-key slice + 32-key
    wing per 128-query tile.
  - exp on scalar engine (scale=1/8, no max subtraction: scores ~ N(0,1));
    band masking applied POST-exp on the bf16 SBUF tiles via gpsimd
    affine_select (two slope-1 predicates main, one wing) — no mask tensor,
    no DVE mask cost, and the PE never runs transposes.
  - ctx^T = [V|1]^T @ P^T per head (M=65): row 64 of the ctx PSUM is the
    softmax denominator l for free. reciprocal (DVE) -> partition_broadcast
    (gpsimd) -> per-column normalize on eviction (DVE tensor_mul).
  - even head evicts straight into ctxT rows 0-63; odd head goes through a
    staging tile + SBUF->SBUF DMA partition shift to rows 64-127.
  - out-proj token-major (lhsT=ctxT, rhs=w_outT), bias added on evict.
  - dense projection / out-proj matmul groups interleaved between attention
    packs to keep the PE HAM-warm end to end.
"""

import os
import sys

sys.path.insert(0, "/opt/trn_rl_repo")

import numpy as np

B, S, D = 8, 1024, 1024
H, HD = 16, 64
P = 128
NT = S // P  # 8 query/token tiles
N_CORES = 8

_CACHE = {}


def _build_nc():
    # bisect switches: V2_BCAST=gps|pe, V2_SEL=pack|chunk,
    # V2_LV: 1=scores+exp+sel only, 2=+ctx (no norm), 3=full
    bcast_mode = os.environ.get("V2_BCAST", "gps")
    sel_mode = os.environ.get("V2_SEL", "pack")
    attn_lv = int(os.environ.get("V2_LV", "3"))
    m65 = os.environ.get("V2_M65", "1") == "1"
    do_shift = os.environ.get("V2_SHIFT", "1") == "1"
    do_wing = os.environ.get("V2_WING", "1") == "1"
    import concourse.bacc as bacc
    import concourse.mybir as mybir
    import concourse.tile as tile

    dt = mybir.dt
    f32, bf16 = dt.float32, dt.bfloat16
    Act = mybir.ActivationFunctionType
    Alu = mybir.AluOpType

    nc = bacc.Bacc("TRN2", target_bir_lowering=False, debug=False,
                   num_devices=N_CORES)

    xt_d = nc.dram_tensor("xt", [D, S], bf16, kind="ExternalInput").ap()
    wi_d = nc.dram_tensor("w_int", [D, 3 * D], bf16, kind="ExternalInput").ap()
    wo_d = nc.dram_tensor("w_outt", [D, D], bf16, kind="ExternalInput").ap()
    bin_d = nc.dram_tensor("b_in_t", [P, 16], f32, kind="ExternalInput").ap()
    bv_d = nc.dram_tensor("bv_bc", [P, D], bf16, kind="ExternalInput").ap()
    bo_d = nc.dram_tensor("bo_bc", [P, D], bf16, kind="ExternalInput").ap()
    out_d = nc.dram_tensor("out", [S, D], f32, kind="ExternalOutput").ap()
    dbg = os.environ.get("V2_DBG", "0") == "1"
    if dbg:
        dbg_p4 = nc.dram_tensor("dbg_p4", [P, 4, P], bf16, kind="ExternalOutput").ap()
        dbg_pw = nc.dram_tensor("dbg_pw", [32, 4, P], bf16, kind="ExternalOutput").ap()
        dbg_r = nc.dram_tensor("dbg_r", [65, 2, P], f32, kind="ExternalOutput").ap()
        dbg_rbc = nc.dram_tensor("dbg_rbc", [64, 2, P], f32, kind="ExternalOutput").ap()
        dbg_ctx = nc.dram_tensor("dbg_ctx", [P, 512], bf16, kind="ExternalOutput").ap()


    with tile.TileContext(nc) as tc:
        with (
            tc.tile_pool(name="const", bufs=1) as cpool,
            tc.tile_pool(name="acts", bufs=1) as apool,
            tc.tile_pool(name="pmain", bufs=4) as pmpool,
            tc.tile_pool(name="pwing", bufs=4) as pwpool,
            tc.tile_pool(name="lr", bufs=2) as lrpool,
            tc.tile_pool(name="r0", bufs=2) as r0pool,
            tc.tile_pool(name="rbc", bufs=2) as rbcpool,
            tc.tile_pool(name="rbc2", bufs=2) as rbc2pool,
            tc.tile_pool(name="stage", bufs=3) as stgpool,
            tc.tile_pool(name="outsb", bufs=2) as outpool,
            tc.tile_pool(name="ps_mm", bufs=2, space="PSUM") as ps_mm,
            tc.tile_pool(name="ps_st", bufs=2, space="PSUM") as ps_st,
            tc.tile_pool(name="ps_wg", bufs=2, space="PSUM") as ps_wg,
            tc.tile_pool(name="ps_pp", bufs=2, space="PSUM") as ps_pp,
        ):
            # ---- persistent SBUF tensors ----
            xt = [cpool.tile([P, S], bf16, tag=f"xt{i}", name=f"xt{i}") for i in range(NT)]
            wi = [cpool.tile([P, 3 * D], bf16, tag=f"wi{i}", name=f"wi{i}") for i in range(NT)]
            wo = [cpool.tile([P, D], bf16, tag=f"wo{i}", name=f"wo{i}") for i in range(NT)]
            bint = cpool.tile([P, 16], f32, tag="bint", name="bint")
            bv = cpool.tile([P, 16, 64], bf16, tag="bv", name="bv")
            bo = cpool.tile([P, D], bf16, tag="bo", name="bo")

            kT = [apool.tile([P, S], bf16, tag=f"kT{h}", name=f"kT{h}") for h in range(H)]
            qT = [apool.tile([P, S], bf16, tag=f"qT{i}", name=f"qT{i}") for i in range(NT)]
            v = [apool.tile([P, 16, 72], bf16, tag=f"v{i}", name=f"v{i}") for i in range(NT)]
            voff = [None] + [apool.tile([P, 16, 72], bf16, tag=f"voff{j}", name=f"voff{j}")
                             for j in range(1, NT + 1)]
            ctxT = [apool.tile([P, S], bf16, tag=f"ctxT{i}", name=f"ctxT{i}") for i in range(NT)]

            zfill = nc.gpsimd.to_reg(0.0)
            ones_r = cpool.tile([65, 2, P], f32, tag="ones_r", name="ones_r")
            nc.vector.memset(ones_r[64:65, :, :], 1.0)
            if attn_lv < 2:
                for i in range(NT):
                    nc.vector.memset(ctxT[i], 0.0)

            def band_select(ap, base, cm, step, reps, width):
                """keep where base + cm*partition + step*inner_idx >= 0"""
                if sel_mode == "pack":
                    nc.gpsimd.affine_select(
                        out=ap, in_=ap, compare_op=Alu.is_ge, fill=zfill,
                        base=base, channel_multiplier=cm,
                        pattern=[[0, reps], [step, width]],
                    )
                else:
                    for c in range(reps):
                        nc.gpsimd.affine_select(
                            out=ap[:, c, :], in_=ap[:, c, :],
                            compare_op=Alu.is_ge, fill=zfill,
                            base=base, channel_multiplier=cm,
                            pattern=[[step, width]],
                        )

            # ---- loads ----
            for i in range(NT):
                nc.sync.dma_start(out=xt[i], in_=xt_d[i * P:(i + 1) * P, :])
                nc.sync.dma_start(out=wi[i][:, 2 * D:3 * D],
                                  in_=wi_d[i * P:(i + 1) * P, 2 * D:3 * D])
            for i in range(NT):
                nc.sync.dma_start(out=wi[i][:, 0:2 * D],
                                  in_=wi_d[i * P:(i + 1) * P, 0:2 * D])
            nc.sync.dma_start(out=bint, in_=bin_d)
            nc.sync.dma_start(out=bv, in_=bv_d)
            for i in range(NT):
                nc.vector.memset(v[i][:, :, 64:72], 1.0)
            for h in range(H):
                nc.gpsimd.memset(kT[h][(1 - h % 2) * 64:(2 - h % 2) * 64, :], 0.0)
            for i in range(NT):
                nc.sync.dma_start(out=wo[i], in_=wo_d[i * P:(i + 1) * P, :])
            nc.sync.dma_start(out=bo, in_=bo_d)

            # ---- projection emitters (interleaved with attention below) ----
            def emit_v(st, nh):
                ps = ps_mm.tile([P, 512], f32, tag="mm", name="mmps")
                for dc in range(NT):
                    nc.tensor.matmul(
                        ps,
                        lhsT=xt[dc][:, st * P:(st + 1) * P],
                        rhs=wi[dc][:, 2 * D + nh * 512: 2 * D + (nh + 1) * 512],
                        start=(dc == 0), stop=(dc == NT - 1),
                    )
                nc.vector.tensor_add(v[st][:, nh * 8:(nh + 1) * 8, 0:64],
                                     ps, bv[:, nh * 8:(nh + 1) * 8, :])

            def emit_voff(j):
                if j < NT:
                    nc.sync.dma_start(out=voff[j][0:16, :, :], in_=v[j - 1][112:128, :, :])
                    nc.sync.dma_start(out=voff[j][16:128, :, :], in_=v[j][0:112, :, :])
                else:
                    nc.vector.memset(voff[NT][:, :, :], 0.0)
                    nc.sync.dma_start(out=voff[NT][0:16, :, :], in_=v[NT - 1][112:128, :, :])

            def emit_kq(split_heads, dst, fbase, bias_col, fc, nh):
                ps = ps_mm.tile([P, 512], f32, tag="mm", name="mmps")
                for dc in range(NT):
                    nc.tensor.matmul(
                        ps,
                        lhsT=wi[dc][:, fbase + fc * P: fbase + (fc + 1) * P],
                        rhs=xt[dc][:, nh * 512:(nh + 1) * 512],
                        start=(dc == 0), stop=(dc == NT - 1),
                    )
                bia = bint[:, bias_col + fc: bias_col + fc + 1]
                if split_heads:
                    for hh in range(2):
                        sl = slice(hh * 64, (hh + 1) * 64)
                        nc.vector.tensor_scalar(
                            out=dst[2 * fc + hh][sl, nh * 512:(nh + 1) * 512],
                            in0=ps[sl, :], scalar1=bia[sl, :],
                            scalar2=None, op0=Alu.add,
                        )
                else:
                    nc.vector.tensor_scalar(
                        out=dst[fc][:, nh * 512:(nh + 1) * 512],
                        in0=ps, scalar1=bia, scalar2=None, op0=Alu.add,
                    )

            # ---- attention per query tile, transpose-free ----
            def attention_tile(t, fillers=()):
                fillers = list(fillers)
                kst = 0 if t == 0 else t * P - 16
                vm = v[0] if t == 0 else voff[t]
                vc = v[1] if t == 0 else voff[t + 1]
                # local band: key j (partition), query i (free); t>0 windows
                # start at tP-16 so allowed is 0<=j-i<=32; t=0 is |j-i|<=16.
                b_lo = 16 if t == 0 else 0    # keep j - i + b_lo >= 0
                b_hi = 16 if t == 0 else 32   # keep i - j + b_hi >= 0
                b_wg = -112 if t == 0 else -96  # wing: keep i - jw + b_wg >= 0
                p4s, pws = [], []
                # t=7's wing has only 16 valid keys: run a 16-row wing
                # matmul, pre-zero the wing P tile, exp/select rows 0:16.
                wrows = 16 if t == NT - 1 else 32
                for pk in range(4):
                    stp = ps_st.tile([P, 4, P], f32, tag="st", name="stps")
                    wgp = ps_wg.tile([32, 4, P], f32, tag="wg", name="wgps")
                    # one start=True per PSUM bank: start marks the whole 2KB
                    # bank row (per written partition) pending-zero, so later
                    # chunks must use start=False (they overwrite their own
                    # pending bytes, accumulate nothing).
                    for hh in range(4):
                        h = pk * 4 + hh
                        nc.tensor.matmul(
                            stp[:, hh, :],
                            lhsT=kT[h][:, kst:kst + 128],
                            rhs=qT[h // 2][:, t * P:(t + 1) * P],
                            start=True, stop=True,
                            skip_group_check=True,
                        )
                        nc.tensor.matmul(
                            wgp[0:wrows, hh, :],
                            lhsT=kT[h][:, kst + 128:kst + 128 + wrows],
                            rhs=qT[h // 2][:, t * P:(t + 1) * P],
                            start=True, stop=True,
                            skip_group_check=True,
                        )
                    p4 = pmpool.tile([P, 4, P], bf16, tag="p4", name="p4")
                    nc.scalar.activation(p4, stp, Act.Exp, scale=0.125)
                    band_select(p4, b_lo, 1, -1, 4, P)
                    band_select(p4, b_hi, -1, 1, 4, P)
                    pw = pwpool.tile([32, 4, P], bf16, tag="pw", name="pw")
                    if wrows < 32:
                        nc.vector.memset(pw, 0.0)
                    nc.scalar.activation(pw[0:wrows, :, :], wgp[0:wrows, :, :],
                                         Act.Exp, scale=0.125)
                    band_select(pw[0:wrows, :, :], b_wg, -1, 1, 4, P)
                    if dbg and t == 0 and pk == 0:
                        nc.sync.dma_start(out=dbg_p4, in_=p4)
                        nc.sync.dma_start(out=dbg_pw, in_=pw)
                    p4s.append(p4)
                    pws.append(pw)
                    if fillers:
                        fillers.pop(0)()
                if attn_lv < 2:
                    while fillers:
                        fillers.pop(0)()
                    return
                M = 65 if m65 else 64
                for hp in range(NT):
                    h0, h1 = 2 * hp, 2 * hp + 1
                    pk, c0 = h0 // 4, h0 % 4
                    pp = ps_pp.tile([65, 2, P], f32, tag="pp", name="pps")
                    for cc, hx in ((0, h0), (1, h1)):
                        nc.tensor.matmul(
                            pp[0:M, cc, :], lhsT=vm[:, hx, 0:M],
                            rhs=p4s[pk][:, c0 + cc, :],
                            start=True, stop=(not do_wing),
                            skip_group_check=True,
                        )
                        if do_wing:
                            nc.tensor.matmul(
                                pp[0:M, cc, :], lhsT=vc[0:32, hx, 0:M],
                                rhs=pws[pk][:, c0 + cc, :],
                                start=False, stop=True, skip_group_check=True,
                            )
                    if attn_lv < 3:
                        nc.vector.tensor_copy(ctxT[hp][0:64, t * P:(t + 1) * P],
                                              pp[0:64, 0, :])
                        stg = stgpool.tile([64, P], bf16, tag="stg", name="stg")
                        nc.vector.tensor_copy(stg, pp[0:64, 1, :])
                        if do_shift:
                            nc.sync.dma_start(
                                out=ctxT[hp][64:128, t * P:(t + 1) * P], in_=stg)
                        if hp % 2 == 1 and fillers:
                            fillers.pop(0)()
                        continue
                    r = lrpool.tile([65, 2, P], f32, tag="r", name="r_t")
                    if dbg and t == 0 and hp == 0:
                        nc.vector.reciprocal(r[64:65, :, :], pp[64:65, :, :])
                        nc.vector.tensor_copy(r[0:64, :, :], pp[0:64, :, :])
                        nc.sync.dma_start(out=dbg_r, in_=r)
                    if bcast_mode == "gps":
                        # l row: psum->sbuf (ACT, idle engine), DMA-shift to
                        # partition 0 (HW partition_broadcast ignores the AP
                        # partition offset), broadcast l, then a WIDE
                        # reciprocal (64 lanes) instead of a 1-lane one.
                        nc.scalar.copy(r[64:65, :, :], pp[64:65, :, :])
                        r0 = r0pool.tile([1, 2, P], f32, tag="r0", name="r0")
                        nc.scalar.dma_start(out=r0, in_=r[64:65, :, :])
                        lbc = rbcpool.tile([64, 2, P], f32, tag="rbc", name="rbc")
                        nc.gpsimd.partition_broadcast(lbc, r0, channels=64)
                        rbc = rbc2pool.tile([64, 2, P], f32, tag="rbc2", name="rbc2")
                        nc.vector.reciprocal(rbc, lbc)
                        if dbg and t == 0 and hp == 0:
                            nc.sync.dma_start(out=dbg_rbc, in_=rbc)
                        rb0, rb1 = rbc[:, 0, :], rbc[:, 1, :]
                    else:
                        # PE broadcast: rbc[p, q] = ones[64] (x) r[q], K=1,
                        # then evict to SBUF (DVE can read only one PSUM input)
                        nc.vector.reciprocal(r[64:65, :, :], pp[64:65, :, :])
                        rbp = ps_st.tile([P, 4, P], f32, tag="st", name="rbcps")
                        nc.tensor.matmul(
                            rbp[0:64, 0:2, :], lhsT=ones_r[64:65, 0, 0:64],
                            rhs=r[64:65, :, :],
                            start=True, stop=True, skip_group_check=True,
                        )
                        rbc = rbcpool.tile([64, 2, P], f32, tag="rbc", name="rbc")
                        nc.scalar.copy(rbc, rbp[0:64, 0:2, :])
                        rb0, rb1 = rbc[:, 0, :], rbc[:, 1, :]
                    nc.vector.tensor_mul(ctxT[hp][0:64, t * P:(t + 1) * P],
                                         pp[0:64, 0, :], rb0)
                    stg = stgpool.tile([64, P], bf16, tag="stg", name="stg")
                    nc.vector.tensor_mul(stg, pp[0:64, 1, :], rb1)
                    if do_shift:
                        nc.scalar.dma_start(
                            out=ctxT[hp][64:128, t * P:(t + 1) * P], in_=stg)
                    if hp % 2 == 1 and fillers:
                        fillers.pop(0)()

            def emit_op(st, nh):
                ps = ps_mm.tile([P, 512], f32, tag="mm", name="mmps")
                for fc in range(NT):
                    nc.tensor.matmul(
                        ps,
                        lhsT=ctxT[fc][:, st * P:(st + 1) * P],
                        rhs=wo[fc][:, nh * 512:(nh + 1) * 512],
                        start=(fc == 0), stop=(fc == NT - 1),
                    )
                o_sb = outpool.tile([P, 512], f32, tag="o", name="o_sb")
                nc.vector.tensor_add(o_sb, ps, bo[:, nh * 512:(nh + 1) * 512])
                nc.sync.dma_start(
                    out=out_d[st * P:(st + 1) * P, nh * 512:(nh + 1) * 512],
                    in_=o_sb)

            # ---- interleaved schedule: dense projection/out-proj groups are
            #      injected between attention packs to keep the PE array warm
            #      and fill cross-engine stalls ----
            def F(fn, *a):
                return lambda: fn(*a)

            for st in range(4):
                for nh in range(2):
                    emit_v(st, nh)
            for j in range(1, 4):
                emit_voff(j)
            for fc in range(NT):
                emit_kq(True, kT, D, 8, fc, 0)
            for fc in range(NT):
                emit_kq(False, qT, 0, 0, fc, 0)

            attention_tile(0, [F(emit_v, 4, 0), F(emit_v, 4, 1),
                               F(emit_kq, True, kT, D, 8, 0, 1),
                               F(emit_kq, True, kT, D, 8, 1, 1)])
            emit_voff(4)
            attention_tile(1, [F(emit_kq, True, kT, D, 8, 2, 1),
                               F(emit_kq, True, kT, D, 8, 3, 1),
                               F(emit_kq, True, kT, D, 8, 4, 1),
                               F(emit_kq, True, kT, D, 8, 5, 1),
                               F(emit_v, 5, 0), F(emit_v, 5, 1)])
            emit_voff(5)
            attention_tile(2, [F(emit_kq, True, kT, D, 8, 6, 1),
                               F(emit_kq, True, kT, D, 8, 7, 1),
                               F(emit_kq, False, qT, 0, 0, 0, 1),
                               F(emit_kq, False, qT, 0, 0, 1, 1),
                               F(emit_v, 6, 0), F(emit_v, 6, 1)])
            emit_voff(6)
            attention_tile(3, [F(emit_kq, False, qT, 0, 0, 2, 1),
                               F(emit_kq, False, qT, 0, 0, 3, 1),
                               F(emit_kq, False, qT, 0, 0, 4, 1),
                               F(emit_kq, False, qT, 0, 0, 5, 1),
                               F(emit_kq, False, qT, 0, 0, 6, 1),
                               F(emit_kq, False, qT, 0, 0, 7, 1),
                               F(emit_v, 7, 0), F(emit_v, 7, 1)])
            emit_voff(7)
            emit_voff(8)
            if dbg:
                nc.sync.dma_start(out=dbg_ctx, in_=ctxT[0][:, 0:512])
            attention_tile(4, [F(emit_op, 0, 0), F(emit_op, 0, 1),
                               F(emit_op, 1, 0), F(emit_op, 1, 1)])
            attention_tile(5, [F(emit_op, 2, 0), F(emit_op, 2, 1),
                               F(emit_op, 3, 0), F(emit_op, 3, 1)])
            attention_tile(6, [F(emit_op, 4, 0), F(emit_op, 4, 1),
                               F(emit_op, 5, 0), F(emit_op, 5, 1)])
            attention_tile(7, [F(emit_op, 6, 0), F(emit_op, 6, 1)])
            emit_op(7, 0)
            emit_op(7, 1)

    nc.compile()
    return nc


def _get_nc():
    if "nc" not in _CACHE:
        _CACHE["nc"] = _build_nc()
    return _CACHE["nc"]


def _prep_inputs(x, w_in, b_in, w_out, b_out, mask):
    import ml_dtypes
    bf16 = ml_dtypes.bfloat16

    x = np.asarray(x, np.float32)
    w_in = np.asarray(w_in, np.float32)
    b_in = np.asarray(b_in, np.float32)
    w_out = np.asarray(w_out, np.float32)
    b_out = np.asarray(b_out, np.float32)

    w_int = np.ascontiguousarray(w_in.T).astype(bf16)          # [D, 3D]
    w_outt = np.ascontiguousarray(w_out.T).astype(bf16)        # [D, D]
    # q,k bias per-partition layout: col c (= global feature chunk), row p
    b_qk = b_in[:2 * D].reshape(16, P).T.astype(np.float32).copy()  # [128,16]
    bv_bc = np.broadcast_to(b_in[2 * D:], (P, D)).astype(bf16).copy()
    bo_bc = np.broadcast_to(b_out, (P, D)).astype(bf16).copy()

    in_maps = []
    for b in range(B):
        xt = np.ascontiguousarray(x[b].T).astype(bf16)         # [D, S]
        in_maps.append({
            "xt": xt, "w_int": w_int, "w_outt": w_outt,
            "b_in_t": b_qk, "bv_bc": bv_bc, "bo_bc": bo_bc,
        })
    return in_maps


def run(x, w_in, b_in, w_out, b_out, mask, trace=False):
    from concourse.bass_utils import run_bass_kernel_spmd
    nc = _get_nc()
    in_maps = _prep_inputs(x, w_in, b_in, w_out, b_out, mask)
    res = run_bass_kernel_spmd(nc, in_maps, list(range(N_CORES)), trace=trace)
    out = np.stack([np.asarray(res.results[b]["out"], np.float32)
                    for b in range(B)])
    return out, res


def kernel(x, w_in, b_in, w_out, b_out, mask):
    out, _ = run(x, w_in, b_in, w_out, b_out, mask)
    return out
